# revision 46
# baseline (speedup 1.0000x reference)
"""GeneAwareContrastive loss — Trainium2 Bass kernel (8 NeuronCores, SPMD).

Cyclic half-strip scheme. G = fn@fn.T is symmetric, so each unordered pair
is computed ONCE: global row-tile t (128 rows, NT=B/128 tiles) computes the
column strip [t*128, t*128 + w(t)*128) mod B with w = NT/2+1 for t < NT/2
and w = NT/2 otherwise. For tile distance D in (0, NT): D < w(t) holds for
exactly one direction of every block pair, so the strips tile the off-
diagonal pairs exactly once (diagonal blocks are computed fully).

Device per core (tiles {4k..4k+3} U {NT/2+4k..+3} - identical program, the
core's rhs is host-rolled by -4k*128 and padded so all strips are static
slices):
  * PE: bf16 matmul G chunks [128, <=1536] into PSUM (2 bufs x 3 banks).
  * ACT: exp(2G) with fused row-sum accum -> per-chunk partials; e values
    written bf16 to SBUF.
  * DVE: max(G, margin/2) with fused row-sum accum (one chunk per 24 runs
    on ACT as relu for engine balance).
  * PE: per 128-col block, a one-hot ones-matmul accumulates column sums of
    the bf16 e values into a persistent [NT, 128] PSUM bank (partition =
    relative block-column); lagged 2 chunks behind the main pass.
Outputs per core: [128, 6T] row-sum partials + [NT, 128] e column sums.

Host: builds bf16 operands, gathers partials, assembles full per-row
sumexp (strip + mirrored column sums), and computes all same-gene /
diagonal-block corrections, the within-pair softplus loss and the cross
loss in float64 from exact per-gene/per-block GEMMs of the same bf16
features (few-MFLOP BLAS).  Pair counts come from the label histogram.
"""

import os
import sys

import numpy as np

sys.path.insert(0, "/opt/trn_rl_repo")

TEMPERATURE = 0.5
W_WITHIN = 1.0
W_CROSS = 0.5
MARGIN = 0.1

N_CORES = 8
CH = 1024  # main column-chunk width (2 PSUM banks)
USE_FP8 = True  # fp8e4m3 DoubleRow main matmuls (bf16 KC-loop if False)

_LAST_RESULT = None
_LAST_RUN = None  # (fn, concat_in, concat_zeros, out_names, out_avals) for timing

_BUILD_CACHE = {}

ACT_RELU_Q = ()  # chunk indices whose relu runs on ACT (relu-form, not max-form)


def _chunks_of(width, ch):
    out = []
    o = 0
    while o < width:
        c = min(ch, width - o)
        out.append((o, c))
        o += c
    return out


def _schedule(T, TL, NT, ch):
    """Flat chunk schedule [(s, rbase, ci, c0, cw, q)], zig-zag (ci-major)
    so early chunks only need the lowest rhs columns."""
    ent = []
    for s in range(T):
        rbase = s if s < TL else NT // 2 + (s - TL)
        width = (NT // 2 + 1) * 128 if s < TL else (NT // 2) * 128
        for ci, (c0, cw) in enumerate(_chunks_of(width, ch)):
            ent.append((ci, s, rbase, c0, cw))
    ent.sort(key=lambda e: (e[0], e[1]))
    return [(s, rbase, ci, c0, cw, q)
            for q, (ci, s, rbase, c0, cw) in enumerate(ent)]


def _build(B, D, ch):
    """Build + compile the per-core Bass/Tile program (identical on all cores)."""
    key = (B, D, ch)
    if key in _BUILD_CACHE:
        return _BUILD_CACHE[key]

    import concourse.bacc as bacc
    import concourse.tile as tile
    import concourse.mybir as mybir

    f32 = mybir.dt.float32
    bf16 = mybir.dt.bfloat16
    fp8 = mybir.dt.float8e4
    mdt = fp8 if USE_FP8 else bf16
    Exp = mybir.ActivationFunctionType.Exp
    Relu = mybir.ActivationFunctionType.Relu
    Alu = mybir.AluOpType
    DR = mybir.MatmulPerfMode.DoubleRow

    KC = D // 128          # contraction chunks
    NT = B // 128          # global row tiles
    T = NT // N_CORES      # row tiles per core
    TL = T // 2            # low (wide) tiles per core
    W1 = (NT // 2 + 1) * 128   # wide strip cols
    W2 = (NT // 2) * 128       # narrow strip cols
    RW = B // 2 + (TL - 1) * 128 + W2  # rel rhs width = (NT/2 + T/2-1)*128 + W2
    # slot s: rel base block rbase = s (s<TL) else NT/2 + (s-TL); width W1/W2
    assert D % 128 == 0 and NT % (2 * N_CORES) == 0 and T % 2 == 0
    assert (not USE_FP8) or KC % 2 == 0

    nc = bacc.Bacc("TRN2", target_bir_lowering=False)

    # flat zig-zag chunk schedule — identical on every core
    sched = _schedule(T, TL, NT, ch)
    n_chunks = len(sched)

    rhs_d = nc.dram_tensor("rhs", [KC, 128, RW], mdt, kind="ExternalInput")
    lhs_d = nc.dram_tensor("lhs", [KC, 128, T * 128], mdt, kind="ExternalInput")
    part_d = nc.dram_tensor("part", [128, 2 * n_chunks], f32, kind="ExternalOutput")
    csum_d = nc.dram_tensor("csum", [n_chunks, ch], f32, kind="ExternalOutput")

    with tile.TileContext(nc) as tc:
        with (
            tc.tile_pool(name="big", bufs=1) as big,
            tc.tile_pool(name="epool", bufs=6) as epool,
            tc.tile_pool(name="rpool", bufs=3) as rpool,
            tc.tile_pool(name="psum", bufs=3, space="PSUM") as psum,
            tc.tile_pool(name="cpsum", bufs=1, space="PSUM") as cpsum,
        ):
            rhs_sb = big.tile([128, KC, RW], mdt)
            lhs_sb = big.tile([128, KC, T * 128], mdt)
            part_sb = big.tile([128, 2 * n_chunks], f32)
            oneh = big.tile([128, 2 * n_chunks + 1], bf16)  # ones at col n_chunks
            nbias = big.tile([128, 1], f32)  # -m/2 bias for the ACT relu chunk
            nc.vector.memset(oneh, 0.0)
            nc.vector.memset(oneh[:, n_chunks : n_chunks + 1], 1.0)
            nc.vector.memset(part_sb, 0.0)
            nc.vector.memset(nbias, -MARGIN / 2)
            csum_ps = cpsum.tile([n_chunks, ch], f32)
            csum_sb = big.tile([n_chunks, ch], f32)

            # lhs + rhs pieces in chunk-consumption order; first pieces small
            emitted = set()
            lhs_done = set()
            for s, rbase, ci, c0, cw, q in sched:
                if s not in lhs_done:
                    lhs_done.add(s)
                    for k in range(KC):
                        nc.sync.dma_start(
                            out=lhs_sb[:, k, s * 128 : (s + 1) * 128],
                            in_=lhs_d[k, :, s * 128 : (s + 1) * 128],
                        )
                a0 = rbase * 128 + c0
                p = (a0 // ch) * ch
                while p < a0 + cw:
                    w = min(ch, RW - p)
                    if p not in emitted:
                        emitted.add(p)
                        step = 512 if len(emitted) <= 2 else w
                        for o in range(0, w, step):
                            ww = min(step, w - o)
                            for k in range(KC):
                                nc.sync.dma_start(
                                    out=rhs_sb[:, k, p + o : p + o + ww],
                                    in_=rhs_d[k, :, p + o : p + o + ww],
                                )
                    p += ch

            # csum row = chunk index, cols = chunk-local offset; first/last
            # chunk touching each 512-piece column range carries start/stop.
            piece_touch = {}
            for s, rbase, ci, c0, cw, q in sched:
                for p0 in range(0, cw, 512):
                    piece_touch.setdefault(p0 // 512, []).append(q)

            pend = []  # pending csum work: (e_tile, q, cw)

            def emit_csum(e_t, q, cw):
                for p0 in range(0, cw, 512):
                    pw = min(512, cw - p0)
                    pi = p0 // 512
                    nc.tensor.matmul(
                        csum_ps[:, p0 : p0 + pw],
                        oneh[:, n_chunks - q : 2 * n_chunks - q],
                        e_t[:, p0 : p0 + pw],
                        start=piece_touch[pi][0] == q,
                        stop=piece_touch[pi][-1] == q,
                        skip_group_check=True,
                    )

            for s, rbase, ci, c0, cw, q in sched:
                a0 = rbase * 128 + c0  # rel col of chunk start
                ps = psum.tile([128, ch], f32, tag="ps")
                for sub0 in range(0, cw, 512):
                    sw = min(512, cw - sub0)
                    if USE_FP8:
                        nc.tensor.matmul(
                            ps[:, sub0 : sub0 + sw],
                            lhs_sb[:, :, s * 128 : (s + 1) * 128],
                            rhs_sb[:, :, a0 + sub0 : a0 + sub0 + sw],
                            start=True,
                            stop=True,
                            perf_mode=DR,
                        )
                    else:
                        for k in range(KC):
                            nc.tensor.matmul(
                                ps[:, sub0 : sub0 + sw],
                                lhs_sb[:, k, s * 128 : (s + 1) * 128],
                                rhs_sb[:, k, a0 + sub0 : a0 + sub0 + sw],
                                start=(k == 0),
                                stop=(k == KC - 1),
                            )
                # lagged csum emission keeps PE fed while exp catches up
                if len(pend) >= 2:
                    emit_csum(*pend.pop(0))
                e_t = epool.tile([128, ch], bf16, tag="e")
                nc.scalar.activation(
                    out=e_t[:, :cw], in_=ps[:, :cw], func=Exp, scale=2.0,
                    accum_out=part_sb[:, q : q + 1],
                )
                r_t = rpool.tile([128, ch], bf16, tag="r")
                if q in ACT_RELU_Q:  # relu chunk on ACT for engine balance
                    nc.scalar.activation(
                        out=r_t[:, :cw], in_=ps[:, :cw], func=Relu,
                        bias=nbias[:, :], scale=1.0,
                        accum_out=part_sb[:, n_chunks + q : n_chunks + q + 1],
                    )
                else:
                    nc.vector.tensor_scalar(
                        out=r_t[:, :cw], in0=ps[:, :cw],
                        scalar1=MARGIN / 2, scalar2=None,
                        op0=Alu.max, op1=Alu.add,
                        accum_out=part_sb[:, n_chunks + q : n_chunks + q + 1],
                    )
                pend.append((e_t, q, cw))
            while pend:
                emit_csum(*pend.pop(0))

            nc.scalar.copy(out=csum_sb, in_=csum_ps)
            nc.sync.dma_start(out=part_d[:, :], in_=part_sb[:])
            nc.sync.dma_start(out=csum_d[:, :], in_=csum_sb[:])

    nc.compile()
    _BUILD_CACHE[key] = (nc, n_chunks, None)
    return _BUILD_CACHE[key]


_RUNNER_CACHE = {}


def _get_runner(key, nc):
    """Build (once) a jitted shard_map callable running the compiled Bass
    program SPMD on the 8 NeuronCores via the axon PJRT backend."""
    if key in _RUNNER_CACHE:
        return _RUNNER_CACHE[key]
    import jax
    from jax.experimental.shard_map import shard_map
    from jax.sharding import Mesh, PartitionSpec
    import concourse.mybir as mybir
    from concourse import bass2jax

    bass2jax.install_neuronx_cc_hook()

    partition_name = nc.partition_id_tensor.name if nc.partition_id_tensor else None
    in_names, out_names, out_avals, zero_outs = [], [], [], []
    for alloc in nc.m.functions[0].allocations:
        if not isinstance(alloc, mybir.MemoryLocationSet):
            continue
        name = alloc.memorylocations[0].name
        if alloc.kind == "ExternalInput":
            if name != partition_name:
                in_names.append(name)
        elif alloc.kind == "ExternalOutput":
            shape = tuple(alloc.tensor_shape)
            dtype = mybir.dt.np(alloc.dtype)
            out_names.append(name)
            out_avals.append(jax.core.ShapedArray(shape, dtype))
            zero_outs.append(np.zeros(shape, dtype))
    n_params = len(in_names)
    n_outs = len(out_avals)
    all_in_names = list(in_names) + list(out_names)
    if partition_name is not None:
        all_in_names.append(partition_name)

    def _body(*args):
        operands = list(args)
        if partition_name is not None:
            operands.append(bass2jax.partition_id_tensor())
        outs = bass2jax._bass_exec_p.bind(
            *operands,
            out_avals=tuple(out_avals),
            in_names=tuple(all_in_names),
            out_names=tuple(out_names),
            lowering_input_output_aliases=(),
            sim_require_finite=True,
            sim_require_nnan=True,
            nc=nc,
        )
        return tuple(outs)

    devices = jax.devices()[:N_CORES]
    mesh = Mesh(np.asarray(devices), ("core",))
    in_specs = (PartitionSpec("core"),) * (n_params + n_outs)
    out_specs = (PartitionSpec("core"),) * n_outs
    donate = tuple(range(n_params, n_params + n_outs))
    fn = jax.jit(
        shard_map(
            _body, mesh=mesh, in_specs=in_specs, out_specs=out_specs, check_rep=False
        ),
        donate_argnums=donate,
        keep_unused=True,
    )
    runner = (fn, in_names, out_names, out_avals, zero_outs)
    _RUNNER_CACHE[key] = runner
    return runner


def _run(nc, key, in_maps):
    """Execute on 8 cores; returns dict name -> stacked [N_CORES, ...] outputs."""
    global _LAST_RUN
    fn, in_names, out_names, out_avals, zero_outs = _get_runner(key, nc)
    concat_in = [
        np.concatenate([in_maps[c][name] for c in range(N_CORES)], axis=0)
        for name in in_names
    ]
    concat_zeros = [
        np.zeros((N_CORES * z.shape[0], *z.shape[1:]), z.dtype) for z in zero_outs
    ]
    _LAST_RUN = (fn, concat_in, concat_zeros, out_names, out_avals)
    out_arrs = fn(*concat_in, *concat_zeros)
    return {
        nm: np.asarray(a).reshape(N_CORES, *out_avals[i].shape)
        for i, (nm, a) in enumerate(zip(out_names, out_arrs))
    }


def _numpy_fallback(features, labs):
    """Direct numpy port of the reference (used only if structure assumptions fail)."""
    B = features.shape[0]
    fn = features / np.linalg.norm(features, axis=1, keepdims=True)
    sim = (fn @ fn.T) / TEMPERATURE
    same = labs[:, None] == labs[None, :]
    eye = np.eye(B, dtype=bool)
    same_off = same & ~eye
    neg = ~same
    has_neg = neg.any(axis=1)
    neg_sim = np.where(neg, sim, -np.inf)
    m = np.max(neg_sim, axis=1)
    m = np.where(np.isfinite(m), m, 0.0)
    lse = m + np.log(np.sum(np.where(neg, np.exp(neg_sim - m[:, None]), 0.0), axis=1))
    lse = np.where(has_neg, lse, 0.0)
    upper = np.triu(np.ones((B, B), dtype=bool), k=1)
    valid = (labs != -1)[:, None]
    pm = same_off & upper & valid & has_neg[:, None]
    z = lse[:, None] - sim
    within = np.where(pm, np.log1p(np.exp(-np.abs(z))) + np.maximum(z, 0), 0.0).sum()
    cross_cnt = int(neg.sum())
    cross_sum = np.where(neg, np.maximum(sim - MARGIN, 0.0), 0.0).sum()
    cross = cross_sum / cross_cnt if cross_cnt > 0 else 0.0
    total = W_WITHIN * within + W_CROSS * cross
    nw = float(same_off.sum())
    idt = np.int64 if labs.dtype == np.int64 else np.int32
    return (
        np.float32(total), np.float32(within), np.float32(cross),
        np.float32(nw), idt(cross_cnt),
    )


def kernel(**inputs):
    global _LAST_RESULT
    import concourse.mybir as mybir

    features = np.asarray(inputs["features"]).astype(np.float32, copy=False)
    labs_in = np.asarray(inputs["gene_labels"])
    labs = labs_in.astype(np.int64)
    B, D = features.shape
    c = MARGIN / 2

    NT = B // 128
    ok = (
        B % 128 == 0
        and D % 128 == 0
        and NT % (2 * N_CORES) == 0
        and (NT // N_CORES) % 2 == 0
        and labs.shape == (B,)
        and np.all(labs >= 0)
    )
    if not ok:
        return _numpy_fallback(features, labs_in)

    T = NT // N_CORES
    TL = T // 2
    KC = D // 128
    W1b = NT // 2 + 1  # wide strip blocks
    W2b = NT // 2
    RW = (NT // 2 + TL - 1) * 128 + W2b * 128

    # ---- host prep: normalize, round to device dtype, per-core rolled operands ----
    norm = np.sqrt((features * features).sum(axis=1, dtype=np.float32))
    with np.errstate(divide="ignore", invalid="ignore"):
        fn = features / norm[:, None]
    bf16 = mybir.dt.np(mybir.dt.bfloat16)
    mdt = mybir.dt.np(mybir.dt.float8e4) if USE_FP8 else bf16
    fnb = fn.astype(mdt)  # the exact operand values the device matmuls see
    fnT = np.ascontiguousarray(fnb.T).reshape(KC, 128, B)

    (nc, n_chunks, _) = _build(B, D, CH)

    in_maps = []
    for k in range(N_CORES):
        idx = (4 * k * 128 + np.arange(RW)) % B
        rhs_c = np.ascontiguousarray(fnT[:, :, idx])
        lhs_cols = []
        for s in range(T):
            rbase = s if s < TL else NT // 2 + (s - TL)
            lhs_cols.append(rhs_c[:, :, rbase * 128 : (rbase + 1) * 128])
        in_maps.append(
            {
                "rhs": rhs_c,
                "lhs": np.ascontiguousarray(np.concatenate(lhs_cols, axis=2)),
            }
        )

    outs = _run(nc, (B, D, CH), in_maps)
    parts = outs["part"]  # [N_CORES, 128, 2*n_chunks]
    csums = outs["csum"]  # [N_CORES, n_chunks, CH]

    # ---- host combine (float64) ----
    # device schedule mirror
    sched = _schedule(T, TL, NT, CH)
    assert len(sched) == n_chunks

    strip_S = np.zeros(B)
    strip_M = np.zeros(B)
    colsum = np.zeros(B)
    for k in range(N_CORES):
        p = parts[k].astype(np.float64)
        cs = csums[k].astype(np.float64)
        for s, rbase, ci, c0, cw, q in sched:
            gt = 4 * k + s if s < TL else NT // 2 + 4 * k + (s - TL)
            rows = slice(gt * 128, (gt + 1) * 128)
            strip_S[rows] += p[:, q]
            strip_M[rows] += p[:, n_chunks + q]
            if q in ACT_RELU_Q:
                # ACT chunks accumulate relu(G-c); max-form needs +c per element
                strip_M[rows] += c * cw
            gc = (4 * k * 128 + rbase * 128 + c0 + np.arange(cw)) % B
            np.add.at(colsum, gc, cs[q, :cw])

    S_total = strip_S + colsum  # full per-row sum of exp(2G) incl. self+same-gene
    # device csum includes each tile's own diagonal block; subtract it exactly
    # (bf16-rounded e values, matching the device SBUF contents)

    fh = fnb.astype(np.float64)

    # diagonal blocks: remove the device-accumulated diag e colsums from
    # S_total (bf16-rounded e values, matching the device SBUF contents) and
    # collect the within/diag max sums for the cross loss.
    W_u = 0.0
    Dg = 0.0
    for t in range(NT):
        idx = np.arange(t * 128, (t + 1) * 128)
        Gg = fh[idx] @ fh[idx].T
        Ed = np.exp(2.0 * Gg).astype(bf16).astype(np.float64)
        S_total[idx] -= Ed.sum(axis=0)
        Mg = np.maximum(Gg, c)
        Dg += np.trace(Mg)
        W_u += (Mg.sum() - np.trace(Mg)) / 2.0

    # same-gene corrections + within loss (exact host GEMMs in f64)
    sneg = S_total.copy()
    order = np.argsort(labs, kind="stable")
    ls = labs[order]
    bounds = np.flatnonzero(np.r_[True, ls[1:] != ls[:-1], True])
    gene_rows = [order[bounds[i] : bounds[i + 1]] for i in range(len(bounds) - 1)]
    sg_relu = 0.0
    n_same_ord = 0
    gene_sims = []
    for idx in gene_rows:
        Gg = fh[idx] @ fh[idx].T
        gene_sims.append(Gg)
        sneg[idx] -= np.exp(2.0 * Gg).sum(axis=1)
        R = np.maximum(Gg - c, 0.0)
        sg_relu += R.sum() - np.maximum(np.diag(Gg) - c, 0.0).sum()
        n_same_ord += len(idx) * (len(idx) - 1)

    has_neg = np.array([B - len(idx) > 0 for idx in gene_rows])
    lse = np.log(np.maximum(sneg, 1e-300))
    within = 0.0
    for gi, idx in enumerate(gene_rows):
        n = len(idx)
        if n < 2 or not has_neg[gi]:
            continue
        sim = 2.0 * gene_sims[gi]
        z = lse[idx][:, None] - sim
        sp = np.logaddexp(0.0, z)
        # pairs i<j in ORIGINAL index order: idx is sorted ascending per gene
        iu = np.triu_indices(n, 1)
        within += sp[iu].sum()

    # cross loss: ordered-pair relu total from strip max sums
    M_dev = strip_M.sum()
    n_ord = B * (B - 1)
    P_relu = 2.0 * (M_dev - Dg - W_u) - c * n_ord
    cross_relu = P_relu - sg_relu
    n_cross = n_ord - n_same_ord
    cross = (2.0 * cross_relu) / n_cross if n_cross > 0 else 0.0

    total = W_WITHIN * within + W_CROSS * cross
    nw = float(n_same_ord)
    idt = np.int64 if labs_in.dtype == np.int64 else np.int32
    return (
        np.float32(total), np.float32(within), np.float32(cross),
        np.float32(nw), idt(n_cross),
    )


# revision 48
# speedup vs baseline: 1.0968x; 1.0968x over previous
"""GeneAwareContrastive loss — Trainium2 Bass kernel (8 NeuronCores, SPMD).

Cyclic half-strip scheme. G = fn@fn.T is symmetric, so each unordered pair
is computed ONCE: global row-tile t (128 rows, NT=B/128 tiles) computes the
column strip [t*128, t*128 + w(t)*128) mod B with w = NT/2+1 for t < NT/2
and w = NT/2 otherwise. For tile distance D in (0, NT): D < w(t) holds for
exactly one direction of every block pair, so the strips tile the off-
diagonal pairs exactly once (diagonal blocks are computed fully).

Device per core (tiles {4k..4k+3} U {NT/2+4k..+3} - identical program, the
core's rhs is host-rolled by -4k*128 and padded so all strips are static
slices):
  * PE: bf16 matmul G chunks [128, <=1536] into PSUM (2 bufs x 3 banks).
  * ACT: exp(2G) with fused row-sum accum -> per-chunk partials; e values
    written bf16 to SBUF.
  * DVE: max(G, margin/2) with fused row-sum accum (one chunk per 24 runs
    on ACT as relu for engine balance).
  * PE: per 128-col block, a one-hot ones-matmul accumulates column sums of
    the bf16 e values into a persistent [NT, 128] PSUM bank (partition =
    relative block-column); lagged 2 chunks behind the main pass.
Outputs per core: [128, 6T] row-sum partials + [NT, 128] e column sums.

Host: builds bf16 operands, gathers partials, assembles full per-row
sumexp (strip + mirrored column sums), and computes all same-gene /
diagonal-block corrections, the within-pair softplus loss and the cross
loss in float64 from exact per-gene/per-block GEMMs of the same bf16
features (few-MFLOP BLAS).  Pair counts come from the label histogram.
"""

import os
import sys

import numpy as np

sys.path.insert(0, "/opt/trn_rl_repo")

TEMPERATURE = 0.5
W_WITHIN = 1.0
W_CROSS = 0.5
MARGIN = 0.1

N_CORES = 8
CH = 1024  # main column-chunk width (2 PSUM banks)
USE_FP8 = True  # fp8e4m3 DoubleRow main matmuls (bf16 KC-loop if False)

_LAST_RESULT = None
_LAST_RUN = None  # (fn, concat_in, concat_zeros, out_names, out_avals) for timing

_BUILD_CACHE = {}

ACT_RELU_Q = ()  # chunk indices whose relu runs on ACT (relu-form, not max-form)


def _chunks_of(width, ch):
    out = []
    o = 0
    while o < width:
        c = min(ch, width - o)
        out.append((o, c))
        o += c
    return out


def _schedule(T, TL, NT, ch):
    """Flat chunk schedule [(s, rbase, ci, c0, cw, q)], zig-zag (ci-major)
    so early chunks only need the lowest rhs columns."""
    ent = []
    for s in range(T):
        rbase = s if s < TL else NT // 2 + (s - TL)
        width = (NT // 2 + 1) * 128 if s < TL else (NT // 2) * 128
        for ci, (c0, cw) in enumerate(_chunks_of(width, ch)):
            ent.append((ci, s, rbase, c0, cw))
    ent.sort(key=lambda e: (0 if e[1] < TL else 1, e[0], e[1]))
    return [(s, rbase, ci, c0, cw, q)
            for q, (ci, s, rbase, c0, cw) in enumerate(ent)]


def _build(B, D, ch):
    """Build + compile the per-core Bass/Tile program (identical on all cores)."""
    key = (B, D, ch)
    if key in _BUILD_CACHE:
        return _BUILD_CACHE[key]

    import concourse.bacc as bacc
    import concourse.tile as tile
    import concourse.mybir as mybir

    f32 = mybir.dt.float32
    bf16 = mybir.dt.bfloat16
    fp8 = mybir.dt.float8e4
    mdt = fp8 if USE_FP8 else bf16
    Exp = mybir.ActivationFunctionType.Exp
    Relu = mybir.ActivationFunctionType.Relu
    Alu = mybir.AluOpType
    DR = mybir.MatmulPerfMode.DoubleRow

    KC = D // 128          # contraction chunks
    NT = B // 128          # global row tiles
    T = NT // N_CORES      # row tiles per core
    TL = T // 2            # low (wide) tiles per core
    W1 = (NT // 2 + 1) * 128   # wide strip cols
    W2 = (NT // 2) * 128       # narrow strip cols
    RW = B // 2 + (TL - 1) * 128 + W2  # rel rhs width = (NT/2 + T/2-1)*128 + W2
    # slot s: rel base block rbase = s (s<TL) else NT/2 + (s-TL); width W1/W2
    assert D % 128 == 0 and NT % (2 * N_CORES) == 0 and T % 2 == 0
    assert (not USE_FP8) or KC % 2 == 0

    nc = bacc.Bacc("TRN2", target_bir_lowering=False)

    # flat zig-zag chunk schedule — identical on every core
    sched = _schedule(T, TL, NT, ch)
    n_chunks = len(sched)

    rhs_d = nc.dram_tensor("rhs", [KC, 128, RW], mdt, kind="ExternalInput")
    lhs_d = nc.dram_tensor("lhs", [KC, 128, T * 128], mdt, kind="ExternalInput")
    part_d = nc.dram_tensor("part", [128, 2 * n_chunks], f32, kind="ExternalOutput")
    csum_d = nc.dram_tensor("csum", [n_chunks, ch], f32, kind="ExternalOutput")

    with tile.TileContext(nc) as tc:
        with (
            tc.tile_pool(name="big", bufs=1) as big,
            tc.tile_pool(name="epool", bufs=6) as epool,
            tc.tile_pool(name="rpool", bufs=3) as rpool,
            tc.tile_pool(name="psum", bufs=3, space="PSUM") as psum,
            tc.tile_pool(name="cpsum", bufs=1, space="PSUM") as cpsum,
        ):
            rhs_sb = big.tile([128, KC, RW], mdt)
            lhs_sb = big.tile([128, KC, T * 128], mdt)
            part_sb = big.tile([128, 2 * n_chunks], f32)
            oneh = big.tile([128, 2 * n_chunks + 1], bf16)  # ones at col n_chunks
            nbias = big.tile([128, 1], f32)  # -m/2 bias for the ACT relu chunk
            nc.vector.memset(oneh, 0.0)
            nc.vector.memset(oneh[:, n_chunks : n_chunks + 1], 1.0)
            nc.vector.memset(part_sb, 0.0)
            nc.vector.memset(nbias, -MARGIN / 2)
            csum_ps = cpsum.tile([n_chunks, ch], f32)
            csum_sb = big.tile([n_chunks, ch], f32)

            # lhs halves (low slots first), then rhs in consumption order
            half = TL * 128
            for h0 in (0, half):
                for k in range(KC):
                    nc.sync.dma_start(
                        out=lhs_sb[:, k, h0 : h0 + half],
                        in_=lhs_d[k, :, h0 : h0 + half],
                    )
            emitted = set()
            for s, rbase, ci, c0, cw, q in sched:
                a0 = rbase * 128 + c0
                p = (a0 // ch) * ch
                while p < a0 + cw:
                    w = min(ch, RW - p)
                    if p not in emitted:
                        emitted.add(p)
                        step = 512 if len(emitted) <= 2 else w
                        for o in range(0, w, step):
                            ww = min(step, w - o)
                            for k in range(KC):
                                nc.sync.dma_start(
                                    out=rhs_sb[:, k, p + o : p + o + ww],
                                    in_=rhs_d[k, :, p + o : p + o + ww],
                                )
                    p += ch

            # csum row = chunk index, cols = chunk-local offset; first/last
            # chunk touching each 512-piece column range carries start/stop.
            piece_touch = {}
            for s, rbase, ci, c0, cw, q in sched:
                for p0 in range(0, cw, 512):
                    piece_touch.setdefault(p0 // 512, []).append(q)

            pend = []  # pending csum work: (e_tile, q, cw)

            def emit_csum(e_t, q, cw):
                for p0 in range(0, cw, 512):
                    pw = min(512, cw - p0)
                    pi = p0 // 512
                    nc.tensor.matmul(
                        csum_ps[:, p0 : p0 + pw],
                        oneh[:, n_chunks - q : 2 * n_chunks - q],
                        e_t[:, p0 : p0 + pw],
                        start=piece_touch[pi][0] == q,
                        stop=piece_touch[pi][-1] == q,
                        skip_group_check=True,
                    )

            for s, rbase, ci, c0, cw, q in sched:
                a0 = rbase * 128 + c0  # rel col of chunk start
                ps = psum.tile([128, ch], f32, tag="ps")
                for sub0 in range(0, cw, 512):
                    sw = min(512, cw - sub0)
                    if USE_FP8:
                        nc.tensor.matmul(
                            ps[:, sub0 : sub0 + sw],
                            lhs_sb[:, :, s * 128 : (s + 1) * 128],
                            rhs_sb[:, :, a0 + sub0 : a0 + sub0 + sw],
                            start=True,
                            stop=True,
                            perf_mode=DR,
                        )
                    else:
                        for k in range(KC):
                            nc.tensor.matmul(
                                ps[:, sub0 : sub0 + sw],
                                lhs_sb[:, k, s * 128 : (s + 1) * 128],
                                rhs_sb[:, k, a0 + sub0 : a0 + sub0 + sw],
                                start=(k == 0),
                                stop=(k == KC - 1),
                            )
                # lagged csum emission keeps PE fed while exp catches up
                if len(pend) >= 2:
                    emit_csum(*pend.pop(0))
                e_t = epool.tile([128, ch], bf16, tag="e")
                nc.scalar.activation(
                    out=e_t[:, :cw], in_=ps[:, :cw], func=Exp, scale=2.0,
                    accum_out=part_sb[:, q : q + 1],
                )
                r_t = rpool.tile([128, ch], bf16, tag="r")
                if q in ACT_RELU_Q:  # relu chunk on ACT for engine balance
                    nc.scalar.activation(
                        out=r_t[:, :cw], in_=ps[:, :cw], func=Relu,
                        bias=nbias[:, :], scale=1.0,
                        accum_out=part_sb[:, n_chunks + q : n_chunks + q + 1],
                    )
                else:
                    nc.vector.tensor_scalar(
                        out=r_t[:, :cw], in0=ps[:, :cw],
                        scalar1=MARGIN / 2, scalar2=None,
                        op0=Alu.max, op1=Alu.add,
                        accum_out=part_sb[:, n_chunks + q : n_chunks + q + 1],
                    )
                pend.append((e_t, q, cw))
            while pend:
                emit_csum(*pend.pop(0))

            nc.scalar.copy(out=csum_sb, in_=csum_ps)
            nc.sync.dma_start(out=part_d[:, :], in_=part_sb[:])
            nc.sync.dma_start(out=csum_d[:, :], in_=csum_sb[:])

    nc.compile()
    _BUILD_CACHE[key] = (nc, n_chunks, None)
    return _BUILD_CACHE[key]


_RUNNER_CACHE = {}


def _get_runner(key, nc):
    """Build (once) a jitted shard_map callable running the compiled Bass
    program SPMD on the 8 NeuronCores via the axon PJRT backend."""
    if key in _RUNNER_CACHE:
        return _RUNNER_CACHE[key]
    import jax
    from jax.experimental.shard_map import shard_map
    from jax.sharding import Mesh, PartitionSpec
    import concourse.mybir as mybir
    from concourse import bass2jax

    bass2jax.install_neuronx_cc_hook()

    partition_name = nc.partition_id_tensor.name if nc.partition_id_tensor else None
    in_names, out_names, out_avals, zero_outs = [], [], [], []
    for alloc in nc.m.functions[0].allocations:
        if not isinstance(alloc, mybir.MemoryLocationSet):
            continue
        name = alloc.memorylocations[0].name
        if alloc.kind == "ExternalInput":
            if name != partition_name:
                in_names.append(name)
        elif alloc.kind == "ExternalOutput":
            shape = tuple(alloc.tensor_shape)
            dtype = mybir.dt.np(alloc.dtype)
            out_names.append(name)
            out_avals.append(jax.core.ShapedArray(shape, dtype))
            zero_outs.append(np.zeros(shape, dtype))
    n_params = len(in_names)
    n_outs = len(out_avals)
    all_in_names = list(in_names) + list(out_names)
    if partition_name is not None:
        all_in_names.append(partition_name)

    def _body(*args):
        operands = list(args)
        if partition_name is not None:
            operands.append(bass2jax.partition_id_tensor())
        outs = bass2jax._bass_exec_p.bind(
            *operands,
            out_avals=tuple(out_avals),
            in_names=tuple(all_in_names),
            out_names=tuple(out_names),
            lowering_input_output_aliases=(),
            sim_require_finite=True,
            sim_require_nnan=True,
            nc=nc,
        )
        return tuple(outs)

    devices = jax.devices()[:N_CORES]
    mesh = Mesh(np.asarray(devices), ("core",))
    in_specs = (PartitionSpec("core"),) * (n_params + n_outs)
    out_specs = (PartitionSpec("core"),) * n_outs
    donate = tuple(range(n_params, n_params + n_outs))
    fn = jax.jit(
        shard_map(
            _body, mesh=mesh, in_specs=in_specs, out_specs=out_specs, check_rep=False
        ),
        donate_argnums=donate,
        keep_unused=True,
    )
    runner = (fn, in_names, out_names, out_avals, zero_outs)
    _RUNNER_CACHE[key] = runner
    return runner


def _run(nc, key, in_maps):
    """Execute on 8 cores; returns dict name -> stacked [N_CORES, ...] outputs."""
    global _LAST_RUN
    fn, in_names, out_names, out_avals, zero_outs = _get_runner(key, nc)
    concat_in = [
        np.concatenate([in_maps[c][name] for c in range(N_CORES)], axis=0)
        for name in in_names
    ]
    concat_zeros = [
        np.zeros((N_CORES * z.shape[0], *z.shape[1:]), z.dtype) for z in zero_outs
    ]
    _LAST_RUN = (fn, concat_in, concat_zeros, out_names, out_avals)
    out_arrs = fn(*concat_in, *concat_zeros)
    return {
        nm: np.asarray(a).reshape(N_CORES, *out_avals[i].shape)
        for i, (nm, a) in enumerate(zip(out_names, out_arrs))
    }


def _numpy_fallback(features, labs):
    """Direct numpy port of the reference (used only if structure assumptions fail)."""
    B = features.shape[0]
    fn = features / np.linalg.norm(features, axis=1, keepdims=True)
    sim = (fn @ fn.T) / TEMPERATURE
    same = labs[:, None] == labs[None, :]
    eye = np.eye(B, dtype=bool)
    same_off = same & ~eye
    neg = ~same
    has_neg = neg.any(axis=1)
    neg_sim = np.where(neg, sim, -np.inf)
    m = np.max(neg_sim, axis=1)
    m = np.where(np.isfinite(m), m, 0.0)
    lse = m + np.log(np.sum(np.where(neg, np.exp(neg_sim - m[:, None]), 0.0), axis=1))
    lse = np.where(has_neg, lse, 0.0)
    upper = np.triu(np.ones((B, B), dtype=bool), k=1)
    valid = (labs != -1)[:, None]
    pm = same_off & upper & valid & has_neg[:, None]
    z = lse[:, None] - sim
    within = np.where(pm, np.log1p(np.exp(-np.abs(z))) + np.maximum(z, 0), 0.0).sum()
    cross_cnt = int(neg.sum())
    cross_sum = np.where(neg, np.maximum(sim - MARGIN, 0.0), 0.0).sum()
    cross = cross_sum / cross_cnt if cross_cnt > 0 else 0.0
    total = W_WITHIN * within + W_CROSS * cross
    nw = float(same_off.sum())
    idt = np.int64 if labs.dtype == np.int64 else np.int32
    return (
        np.float32(total), np.float32(within), np.float32(cross),
        np.float32(nw), idt(cross_cnt),
    )


def kernel(**inputs):
    global _LAST_RESULT
    import concourse.mybir as mybir

    features = np.asarray(inputs["features"]).astype(np.float32, copy=False)
    labs_in = np.asarray(inputs["gene_labels"])
    labs = labs_in.astype(np.int64)
    B, D = features.shape
    c = MARGIN / 2

    NT = B // 128
    ok = (
        B % 128 == 0
        and D % 128 == 0
        and NT % (2 * N_CORES) == 0
        and (NT // N_CORES) % 2 == 0
        and labs.shape == (B,)
        and np.all(labs >= 0)
    )
    if not ok:
        return _numpy_fallback(features, labs_in)

    T = NT // N_CORES
    TL = T // 2
    KC = D // 128
    W1b = NT // 2 + 1  # wide strip blocks
    W2b = NT // 2
    RW = (NT // 2 + TL - 1) * 128 + W2b * 128

    # ---- host prep: normalize, round to device dtype, per-core rolled operands ----
    norm = np.sqrt((features * features).sum(axis=1, dtype=np.float32))
    with np.errstate(divide="ignore", invalid="ignore"):
        fn = features / norm[:, None]
    bf16 = mybir.dt.np(mybir.dt.bfloat16)
    mdt = mybir.dt.np(mybir.dt.float8e4) if USE_FP8 else bf16
    fnb = fn.astype(mdt)  # the exact operand values the device matmuls see
    fnT = np.ascontiguousarray(fnb.T).reshape(KC, 128, B)

    (nc, n_chunks, _) = _build(B, D, CH)

    in_maps = []
    for k in range(N_CORES):
        idx = (4 * k * 128 + np.arange(RW)) % B
        rhs_c = np.ascontiguousarray(fnT[:, :, idx])
        lhs_cols = []
        for s in range(T):
            rbase = s if s < TL else NT // 2 + (s - TL)
            lhs_cols.append(rhs_c[:, :, rbase * 128 : (rbase + 1) * 128])
        in_maps.append(
            {
                "rhs": rhs_c,
                "lhs": np.ascontiguousarray(np.concatenate(lhs_cols, axis=2)),
            }
        )

    outs = _run(nc, (B, D, CH), in_maps)
    parts = outs["part"]  # [N_CORES, 128, 2*n_chunks]
    csums = outs["csum"]  # [N_CORES, n_chunks, CH]

    # ---- host combine (float64) ----
    # device schedule mirror
    sched = _schedule(T, TL, NT, CH)
    assert len(sched) == n_chunks

    strip_S = np.zeros(B)
    strip_M = np.zeros(B)
    colsum = np.zeros(B)
    for k in range(N_CORES):
        p = parts[k].astype(np.float64)
        cs = csums[k].astype(np.float64)
        for s, rbase, ci, c0, cw, q in sched:
            gt = 4 * k + s if s < TL else NT // 2 + 4 * k + (s - TL)
            rows = slice(gt * 128, (gt + 1) * 128)
            strip_S[rows] += p[:, q]
            strip_M[rows] += p[:, n_chunks + q]
            if q in ACT_RELU_Q:
                # ACT chunks accumulate relu(G-c); max-form needs +c per element
                strip_M[rows] += c * cw
            gc = (4 * k * 128 + rbase * 128 + c0 + np.arange(cw)) % B
            np.add.at(colsum, gc, cs[q, :cw])

    S_total = strip_S + colsum  # full per-row sum of exp(2G) incl. self+same-gene
    # device csum includes each tile's own diagonal block; subtract it exactly
    # (bf16-rounded e values, matching the device SBUF contents)

    fh = fnb.astype(np.float64)

    # diagonal blocks: remove the device-accumulated diag e colsums from
    # S_total (bf16-rounded e values, matching the device SBUF contents) and
    # collect the within/diag max sums for the cross loss.
    W_u = 0.0
    Dg = 0.0
    for t in range(NT):
        idx = np.arange(t * 128, (t + 1) * 128)
        Gg = fh[idx] @ fh[idx].T
        Ed = np.exp(2.0 * Gg).astype(bf16).astype(np.float64)
        S_total[idx] -= Ed.sum(axis=0)
        Mg = np.maximum(Gg, c)
        Dg += np.trace(Mg)
        W_u += (Mg.sum() - np.trace(Mg)) / 2.0

    # same-gene corrections + within loss (exact host GEMMs in f64)
    sneg = S_total.copy()
    order = np.argsort(labs, kind="stable")
    ls = labs[order]
    bounds = np.flatnonzero(np.r_[True, ls[1:] != ls[:-1], True])
    gene_rows = [order[bounds[i] : bounds[i + 1]] for i in range(len(bounds) - 1)]
    sg_relu = 0.0
    n_same_ord = 0
    gene_sims = []
    for idx in gene_rows:
        Gg = fh[idx] @ fh[idx].T
        gene_sims.append(Gg)
        sneg[idx] -= np.exp(2.0 * Gg).sum(axis=1)
        R = np.maximum(Gg - c, 0.0)
        sg_relu += R.sum() - np.maximum(np.diag(Gg) - c, 0.0).sum()
        n_same_ord += len(idx) * (len(idx) - 1)

    has_neg = np.array([B - len(idx) > 0 for idx in gene_rows])
    lse = np.log(np.maximum(sneg, 1e-300))
    within = 0.0
    for gi, idx in enumerate(gene_rows):
        n = len(idx)
        if n < 2 or not has_neg[gi]:
            continue
        sim = 2.0 * gene_sims[gi]
        z = lse[idx][:, None] - sim
        sp = np.logaddexp(0.0, z)
        # pairs i<j in ORIGINAL index order: idx is sorted ascending per gene
        iu = np.triu_indices(n, 1)
        within += sp[iu].sum()

    # cross loss: ordered-pair relu total from strip max sums
    M_dev = strip_M.sum()
    n_ord = B * (B - 1)
    P_relu = 2.0 * (M_dev - Dg - W_u) - c * n_ord
    cross_relu = P_relu - sg_relu
    n_cross = n_ord - n_same_ord
    cross = (2.0 * cross_relu) / n_cross if n_cross > 0 else 0.0

    total = W_WITHIN * within + W_CROSS * cross
    nw = float(n_same_ord)
    idt = np.int64 if labs_in.dtype == np.int64 else np.int32
    return (
        np.float32(total), np.float32(within), np.float32(cross),
        np.float32(nw), idt(n_cross),
    )


# revision 49
# speedup vs baseline: 1.1158x; 1.0173x over previous
"""GeneAwareContrastive loss — Trainium2 Bass kernel (8 NeuronCores, SPMD).

Cyclic half-strip scheme. G = fn@fn.T is symmetric, so each unordered pair
is computed ONCE: global row-tile t (128 rows, NT=B/128 tiles) computes the
column strip [t*128, t*128 + w(t)*128) mod B with w = NT/2+1 for t < NT/2
and w = NT/2 otherwise. For tile distance D in (0, NT): D < w(t) holds for
exactly one direction of every block pair, so the strips tile the off-
diagonal pairs exactly once (diagonal blocks are computed fully).

Device per core (tiles {4k..4k+3} U {NT/2+4k..+3} - identical program, the
core's rhs is host-rolled by -4k*128 and padded so all strips are static
slices):
  * PE: bf16 matmul G chunks [128, <=1536] into PSUM (2 bufs x 3 banks).
  * ACT: exp(2G) with fused row-sum accum -> per-chunk partials; e values
    written bf16 to SBUF.
  * DVE: max(G, margin/2) with fused row-sum accum (one chunk per 24 runs
    on ACT as relu for engine balance).
  * PE: per 128-col block, a one-hot ones-matmul accumulates column sums of
    the bf16 e values into a persistent [NT, 128] PSUM bank (partition =
    relative block-column); lagged 2 chunks behind the main pass.
Outputs per core: [128, 6T] row-sum partials + [NT, 128] e column sums.

Host: builds bf16 operands, gathers partials, assembles full per-row
sumexp (strip + mirrored column sums), and computes all same-gene /
diagonal-block corrections, the within-pair softplus loss and the cross
loss in float64 from exact per-gene/per-block GEMMs of the same bf16
features (few-MFLOP BLAS).  Pair counts come from the label histogram.
"""

import os
import sys

import numpy as np

sys.path.insert(0, "/opt/trn_rl_repo")

TEMPERATURE = 0.5
W_WITHIN = 1.0
W_CROSS = 0.5
MARGIN = 0.1

N_CORES = 8
CH = 1024  # main column-chunk width (2 PSUM banks)
USE_FP8 = True  # fp8e4m3 DoubleRow main matmuls (bf16 KC-loop if False)

_LAST_RESULT = None
_LAST_RUN = None  # (fn, concat_in, concat_zeros, out_names, out_avals) for timing

_BUILD_CACHE = {}

ACT_RELU_Q = ()  # chunk indices whose relu runs on ACT (relu-form, not max-form)


def _chunks_of(width, ch):
    out = []
    o = 0
    while o < width:
        c = min(ch, width - o)
        out.append((o, c))
        o += c
    return out


def _schedule(T, TL, NT, ch):
    """Flat chunk schedule [(s, rbase, ci, c0, cw, q)], zig-zag (ci-major)
    so early chunks only need the lowest rhs columns."""
    ent = []
    for s in range(T):
        rbase = s if s < TL else NT // 2 + (s - TL)
        width = (NT // 2 + 1) * 128 if s < TL else (NT // 2) * 128
        for ci, (c0, cw) in enumerate(_chunks_of(width, ch)):
            ent.append((ci, s, rbase, c0, cw))
    ent.sort(key=lambda e: (e[0], e[1]))
    return [(s, rbase, ci, c0, cw, q)
            for q, (ci, s, rbase, c0, cw) in enumerate(ent)]


def _build(B, D, ch):
    """Build + compile the per-core Bass/Tile program (identical on all cores)."""
    key = (B, D, ch)
    if key in _BUILD_CACHE:
        return _BUILD_CACHE[key]

    import concourse.bacc as bacc
    import concourse.tile as tile
    import concourse.mybir as mybir

    f32 = mybir.dt.float32
    bf16 = mybir.dt.bfloat16
    fp8 = mybir.dt.float8e4
    mdt = fp8 if USE_FP8 else bf16
    Exp = mybir.ActivationFunctionType.Exp
    Relu = mybir.ActivationFunctionType.Relu
    Alu = mybir.AluOpType
    DR = mybir.MatmulPerfMode.DoubleRow

    KC = D // 128          # contraction chunks
    NT = B // 128          # global row tiles
    T = NT // N_CORES      # row tiles per core
    TL = T // 2            # low (wide) tiles per core
    W1 = (NT // 2 + 1) * 128   # wide strip cols
    W2 = (NT // 2) * 128       # narrow strip cols
    RW = B // 2 + (TL - 1) * 128 + W2  # rel rhs width = (NT/2 + T/2-1)*128 + W2
    # slot s: rel base block rbase = s (s<TL) else NT/2 + (s-TL); width W1/W2
    assert D % 128 == 0 and NT % (2 * N_CORES) == 0 and T % 2 == 0
    assert (not USE_FP8) or KC % 2 == 0

    nc = bacc.Bacc("TRN2", target_bir_lowering=False)

    # flat zig-zag chunk schedule — identical on every core
    sched = _schedule(T, TL, NT, ch)
    n_chunks = len(sched)

    rhs_d = nc.dram_tensor("rhs", [KC, 128, RW], mdt, kind="ExternalInput")
    lhs_d = nc.dram_tensor("lhs", [KC, 128, T * 128], mdt, kind="ExternalInput")
    part_d = nc.dram_tensor("part", [128, 2 * n_chunks], f32, kind="ExternalOutput")
    csum_d = nc.dram_tensor("csum", [n_chunks, ch], f32, kind="ExternalOutput")

    with tile.TileContext(nc) as tc:
        with (
            tc.tile_pool(name="big", bufs=1) as big,
            tc.tile_pool(name="epool", bufs=6) as epool,
            tc.tile_pool(name="rpool", bufs=3) as rpool,
            tc.tile_pool(name="psum", bufs=3, space="PSUM") as psum,
            tc.tile_pool(name="cpsum", bufs=1, space="PSUM") as cpsum,
        ):
            rhs_sb = big.tile([128, KC, RW], mdt)
            lhs_sb = big.tile([128, KC, T * 128], mdt)
            part_sb = big.tile([128, 2 * n_chunks], f32)
            oneh = big.tile([128, 2 * n_chunks + 1], bf16)  # ones at col n_chunks
            nbias = big.tile([128, 1], f32)  # -m/2 bias for the ACT relu chunk
            nc.vector.memset(oneh, 0.0)
            nc.vector.memset(oneh[:, n_chunks : n_chunks + 1], 1.0)
            nc.vector.memset(part_sb, 0.0)
            nc.vector.memset(nbias, -MARGIN / 2)
            csum_ps = cpsum.tile([n_chunks, ch], f32)
            csum_sb = big.tile([n_chunks, ch], f32)

            # lhs halves (low slots first), then rhs in consumption order
            half = TL * 128
            for h0 in (0, half):
                for k in range(KC):
                    nc.sync.dma_start(
                        out=lhs_sb[:, k, h0 : h0 + half],
                        in_=lhs_d[k, :, h0 : h0 + half],
                    )
            emitted = set()
            for s, rbase, ci, c0, cw, q in sched:
                a0 = rbase * 128 + c0
                p = (a0 // ch) * ch
                while p < a0 + cw:
                    w = min(ch, RW - p)
                    if p not in emitted:
                        emitted.add(p)
                        step = 512 if len(emitted) <= 2 else w
                        for o in range(0, w, step):
                            ww = min(step, w - o)
                            for k in range(KC):
                                nc.sync.dma_start(
                                    out=rhs_sb[:, k, p + o : p + o + ww],
                                    in_=rhs_d[k, :, p + o : p + o + ww],
                                )
                    p += ch

            # csum row = chunk index, cols = chunk-local offset; first/last
            # chunk touching each 512-piece column range carries start/stop.
            piece_touch = {}
            for s, rbase, ci, c0, cw, q in sched:
                for p0 in range(0, cw, 512):
                    piece_touch.setdefault(p0 // 512, []).append(q)

            pend = []  # pending csum work: (e_tile, q, cw)

            def emit_csum(e_t, q, cw):
                for p0 in range(0, cw, 512):
                    pw = min(512, cw - p0)
                    pi = p0 // 512
                    nc.tensor.matmul(
                        csum_ps[:, p0 : p0 + pw],
                        oneh[:, n_chunks - q : 2 * n_chunks - q],
                        e_t[:, p0 : p0 + pw],
                        start=piece_touch[pi][0] == q,
                        stop=piece_touch[pi][-1] == q,
                        skip_group_check=True,
                    )

            for s, rbase, ci, c0, cw, q in sched:
                a0 = rbase * 128 + c0  # rel col of chunk start
                ps = psum.tile([128, ch], f32, tag="ps")
                for sub0 in range(0, cw, 512):
                    sw = min(512, cw - sub0)
                    if USE_FP8:
                        nc.tensor.matmul(
                            ps[:, sub0 : sub0 + sw],
                            lhs_sb[:, :, s * 128 : (s + 1) * 128],
                            rhs_sb[:, :, a0 + sub0 : a0 + sub0 + sw],
                            start=True,
                            stop=True,
                            perf_mode=DR,
                        )
                    else:
                        for k in range(KC):
                            nc.tensor.matmul(
                                ps[:, sub0 : sub0 + sw],
                                lhs_sb[:, k, s * 128 : (s + 1) * 128],
                                rhs_sb[:, k, a0 + sub0 : a0 + sub0 + sw],
                                start=(k == 0),
                                stop=(k == KC - 1),
                            )
                # lagged csum emission keeps PE fed while exp catches up
                if len(pend) >= 2:
                    emit_csum(*pend.pop(0))
                e_t = epool.tile([128, ch], bf16, tag="e")
                nc.scalar.activation(
                    out=e_t[:, :cw], in_=ps[:, :cw], func=Exp, scale=2.0,
                    accum_out=part_sb[:, q : q + 1],
                )
                r_t = rpool.tile([128, ch], bf16, tag="r")
                if q in ACT_RELU_Q:  # relu chunk on ACT for engine balance
                    nc.scalar.activation(
                        out=r_t[:, :cw], in_=ps[:, :cw], func=Relu,
                        bias=nbias[:, :], scale=1.0,
                        accum_out=part_sb[:, n_chunks + q : n_chunks + q + 1],
                    )
                else:
                    nc.vector.tensor_scalar(
                        out=r_t[:, :cw], in0=ps[:, :cw],
                        scalar1=MARGIN / 2, scalar2=None,
                        op0=Alu.max, op1=Alu.add,
                        accum_out=part_sb[:, n_chunks + q : n_chunks + q + 1],
                    )
                pend.append((e_t, q, cw))
            while pend:
                emit_csum(*pend.pop(0))

            nc.scalar.copy(out=csum_sb, in_=csum_ps)
            nc.sync.dma_start(out=part_d[:, :], in_=part_sb[:])
            nc.sync.dma_start(out=csum_d[:, :], in_=csum_sb[:])

    nc.compile()
    _BUILD_CACHE[key] = (nc, n_chunks, None)
    return _BUILD_CACHE[key]


_RUNNER_CACHE = {}


def _get_runner(key, nc):
    """Build (once) a jitted shard_map callable running the compiled Bass
    program SPMD on the 8 NeuronCores via the axon PJRT backend."""
    if key in _RUNNER_CACHE:
        return _RUNNER_CACHE[key]
    import jax
    from jax.experimental.shard_map import shard_map
    from jax.sharding import Mesh, PartitionSpec
    import concourse.mybir as mybir
    from concourse import bass2jax

    bass2jax.install_neuronx_cc_hook()

    partition_name = nc.partition_id_tensor.name if nc.partition_id_tensor else None
    in_names, out_names, out_avals, zero_outs = [], [], [], []
    for alloc in nc.m.functions[0].allocations:
        if not isinstance(alloc, mybir.MemoryLocationSet):
            continue
        name = alloc.memorylocations[0].name
        if alloc.kind == "ExternalInput":
            if name != partition_name:
                in_names.append(name)
        elif alloc.kind == "ExternalOutput":
            shape = tuple(alloc.tensor_shape)
            dtype = mybir.dt.np(alloc.dtype)
            out_names.append(name)
            out_avals.append(jax.core.ShapedArray(shape, dtype))
            zero_outs.append(np.zeros(shape, dtype))
    n_params = len(in_names)
    n_outs = len(out_avals)
    all_in_names = list(in_names) + list(out_names)
    if partition_name is not None:
        all_in_names.append(partition_name)

    def _body(*args):
        operands = list(args)
        if partition_name is not None:
            operands.append(bass2jax.partition_id_tensor())
        outs = bass2jax._bass_exec_p.bind(
            *operands,
            out_avals=tuple(out_avals),
            in_names=tuple(all_in_names),
            out_names=tuple(out_names),
            lowering_input_output_aliases=(),
            sim_require_finite=True,
            sim_require_nnan=True,
            nc=nc,
        )
        return tuple(outs)

    devices = jax.devices()[:N_CORES]
    mesh = Mesh(np.asarray(devices), ("core",))
    in_specs = (PartitionSpec("core"),) * (n_params + n_outs)
    out_specs = (PartitionSpec("core"),) * n_outs
    donate = tuple(range(n_params, n_params + n_outs))
    fn = jax.jit(
        shard_map(
            _body, mesh=mesh, in_specs=in_specs, out_specs=out_specs, check_rep=False
        ),
        donate_argnums=donate,
        keep_unused=True,
    )
    runner = (fn, in_names, out_names, out_avals, zero_outs)
    _RUNNER_CACHE[key] = runner
    return runner


def _run(nc, key, in_maps):
    """Execute on 8 cores; returns dict name -> stacked [N_CORES, ...] outputs."""
    global _LAST_RUN
    fn, in_names, out_names, out_avals, zero_outs = _get_runner(key, nc)
    concat_in = [
        np.concatenate([in_maps[c][name] for c in range(N_CORES)], axis=0)
        for name in in_names
    ]
    concat_zeros = [
        np.zeros((N_CORES * z.shape[0], *z.shape[1:]), z.dtype) for z in zero_outs
    ]
    _LAST_RUN = (fn, concat_in, concat_zeros, out_names, out_avals)
    out_arrs = fn(*concat_in, *concat_zeros)
    return {
        nm: np.asarray(a).reshape(N_CORES, *out_avals[i].shape)
        for i, (nm, a) in enumerate(zip(out_names, out_arrs))
    }


def _numpy_fallback(features, labs):
    """Direct numpy port of the reference (used only if structure assumptions fail)."""
    B = features.shape[0]
    fn = features / np.linalg.norm(features, axis=1, keepdims=True)
    sim = (fn @ fn.T) / TEMPERATURE
    same = labs[:, None] == labs[None, :]
    eye = np.eye(B, dtype=bool)
    same_off = same & ~eye
    neg = ~same
    has_neg = neg.any(axis=1)
    neg_sim = np.where(neg, sim, -np.inf)
    m = np.max(neg_sim, axis=1)
    m = np.where(np.isfinite(m), m, 0.0)
    lse = m + np.log(np.sum(np.where(neg, np.exp(neg_sim - m[:, None]), 0.0), axis=1))
    lse = np.where(has_neg, lse, 0.0)
    upper = np.triu(np.ones((B, B), dtype=bool), k=1)
    valid = (labs != -1)[:, None]
    pm = same_off & upper & valid & has_neg[:, None]
    z = lse[:, None] - sim
    within = np.where(pm, np.log1p(np.exp(-np.abs(z))) + np.maximum(z, 0), 0.0).sum()
    cross_cnt = int(neg.sum())
    cross_sum = np.where(neg, np.maximum(sim - MARGIN, 0.0), 0.0).sum()
    cross = cross_sum / cross_cnt if cross_cnt > 0 else 0.0
    total = W_WITHIN * within + W_CROSS * cross
    nw = float(same_off.sum())
    idt = np.int64 if labs.dtype == np.int64 else np.int32
    return (
        np.float32(total), np.float32(within), np.float32(cross),
        np.float32(nw), idt(cross_cnt),
    )


def kernel(**inputs):
    global _LAST_RESULT
    import concourse.mybir as mybir

    features = np.asarray(inputs["features"]).astype(np.float32, copy=False)
    labs_in = np.asarray(inputs["gene_labels"])
    labs = labs_in.astype(np.int64)
    B, D = features.shape
    c = MARGIN / 2

    NT = B // 128
    ok = (
        B % 128 == 0
        and D % 128 == 0
        and NT % (2 * N_CORES) == 0
        and (NT // N_CORES) % 2 == 0
        and labs.shape == (B,)
        and np.all(labs >= 0)
    )
    if not ok:
        return _numpy_fallback(features, labs_in)

    T = NT // N_CORES
    TL = T // 2
    KC = D // 128
    W1b = NT // 2 + 1  # wide strip blocks
    W2b = NT // 2
    RW = (NT // 2 + TL - 1) * 128 + W2b * 128

    # ---- host prep: normalize, round to device dtype, per-core rolled operands ----
    norm = np.sqrt((features * features).sum(axis=1, dtype=np.float32))
    with np.errstate(divide="ignore", invalid="ignore"):
        fn = features / norm[:, None]
    bf16 = mybir.dt.np(mybir.dt.bfloat16)
    mdt = mybir.dt.np(mybir.dt.float8e4) if USE_FP8 else bf16
    fnb = fn.astype(mdt)  # the exact operand values the device matmuls see
    fnT = np.ascontiguousarray(fnb.T).reshape(KC, 128, B)

    (nc, n_chunks, _) = _build(B, D, CH)

    in_maps = []
    for k in range(N_CORES):
        idx = (4 * k * 128 + np.arange(RW)) % B
        rhs_c = np.ascontiguousarray(fnT[:, :, idx])
        lhs_cols = []
        for s in range(T):
            rbase = s if s < TL else NT // 2 + (s - TL)
            lhs_cols.append(rhs_c[:, :, rbase * 128 : (rbase + 1) * 128])
        in_maps.append(
            {
                "rhs": rhs_c,
                "lhs": np.ascontiguousarray(np.concatenate(lhs_cols, axis=2)),
            }
        )

    outs = _run(nc, (B, D, CH), in_maps)
    parts = outs["part"]  # [N_CORES, 128, 2*n_chunks]
    csums = outs["csum"]  # [N_CORES, n_chunks, CH]

    # ---- host combine (float64) ----
    # device schedule mirror
    sched = _schedule(T, TL, NT, CH)
    assert len(sched) == n_chunks

    strip_S = np.zeros(B)
    strip_M = np.zeros(B)
    colsum = np.zeros(B)
    for k in range(N_CORES):
        p = parts[k].astype(np.float64)
        cs = csums[k].astype(np.float64)
        for s, rbase, ci, c0, cw, q in sched:
            gt = 4 * k + s if s < TL else NT // 2 + 4 * k + (s - TL)
            rows = slice(gt * 128, (gt + 1) * 128)
            strip_S[rows] += p[:, q]
            strip_M[rows] += p[:, n_chunks + q]
            if q in ACT_RELU_Q:
                # ACT chunks accumulate relu(G-c); max-form needs +c per element
                strip_M[rows] += c * cw
            gc = (4 * k * 128 + rbase * 128 + c0 + np.arange(cw)) % B
            np.add.at(colsum, gc, cs[q, :cw])

    S_total = strip_S + colsum  # full per-row sum of exp(2G) incl. self+same-gene
    # device csum includes each tile's own diagonal block; subtract it exactly
    # (bf16-rounded e values, matching the device SBUF contents)

    fh = fnb.astype(np.float64)

    # diagonal blocks: remove the device-accumulated diag e colsums from
    # S_total (bf16-rounded e values, matching the device SBUF contents) and
    # collect the within/diag max sums for the cross loss.
    W_u = 0.0
    Dg = 0.0
    for t in range(NT):
        idx = np.arange(t * 128, (t + 1) * 128)
        Gg = fh[idx] @ fh[idx].T
        Ed = np.exp(2.0 * Gg).astype(bf16).astype(np.float64)
        S_total[idx] -= Ed.sum(axis=0)
        Mg = np.maximum(Gg, c)
        Dg += np.trace(Mg)
        W_u += (Mg.sum() - np.trace(Mg)) / 2.0

    # same-gene corrections + within loss (exact host GEMMs in f64)
    sneg = S_total.copy()
    order = np.argsort(labs, kind="stable")
    ls = labs[order]
    bounds = np.flatnonzero(np.r_[True, ls[1:] != ls[:-1], True])
    gene_rows = [order[bounds[i] : bounds[i + 1]] for i in range(len(bounds) - 1)]
    sg_relu = 0.0
    n_same_ord = 0
    gene_sims = []
    for idx in gene_rows:
        Gg = fh[idx] @ fh[idx].T
        gene_sims.append(Gg)
        sneg[idx] -= np.exp(2.0 * Gg).sum(axis=1)
        R = np.maximum(Gg - c, 0.0)
        sg_relu += R.sum() - np.maximum(np.diag(Gg) - c, 0.0).sum()
        n_same_ord += len(idx) * (len(idx) - 1)

    has_neg = np.array([B - len(idx) > 0 for idx in gene_rows])
    lse = np.log(np.maximum(sneg, 1e-300))
    within = 0.0
    for gi, idx in enumerate(gene_rows):
        n = len(idx)
        if n < 2 or not has_neg[gi]:
            continue
        sim = 2.0 * gene_sims[gi]
        z = lse[idx][:, None] - sim
        sp = np.logaddexp(0.0, z)
        # pairs i<j in ORIGINAL index order: idx is sorted ascending per gene
        iu = np.triu_indices(n, 1)
        within += sp[iu].sum()

    # cross loss: ordered-pair relu total from strip max sums
    M_dev = strip_M.sum()
    n_ord = B * (B - 1)
    P_relu = 2.0 * (M_dev - Dg - W_u) - c * n_ord
    cross_relu = P_relu - sg_relu
    n_cross = n_ord - n_same_ord
    cross = (2.0 * cross_relu) / n_cross if n_cross > 0 else 0.0

    total = W_WITHIN * within + W_CROSS * cross
    nw = float(n_same_ord)
    idt = np.int64 if labs_in.dtype == np.int64 else np.int32
    return (
        np.float32(total), np.float32(within), np.float32(cross),
        np.float32(nw), idt(n_cross),
    )
